# revision 9
# baseline (speedup 1.0000x reference)
"""Block-sparse top-k masked linear for Trainium2, tensor-parallel over 8 cores.

out = (block_masked x) @ W + bias
  x: (128, 1, 4096) fp16, W: (4096, 11008) fp16, bias: (11008,) fp16
  mask: per (32-row x 64-col) block of x, keep blocks whose mean |x| is
  >= the 32nd-largest of the 64 k-block activations in that row block.

Sharding: column-parallel — each of the 8 cores gets an 11008/8 = 1376
column slice of W and bias; x is replicated; outputs are concatenated.

Schedule: x chunks dispatch first on the two HWDGE queues (they gate the
top-k mask), then one packed f16 const DMA (ident|ET|bias) + E, then the
W stream in big chunks. W is pre-packed on the host to (128, 32*1376):
partition p holds row kt*128+p of every k-tile contiguously, so W chunks
move multi-KB contiguous runs per partition (full HBM rate). The mask is
applied to x rows (per partition-block, per 64-col k-block) before the
PE transposes that feed the GEMM.
"""
from contextlib import ExitStack

import numpy as np

import concourse.bass as bass
import concourse.tile as tile
from concourse import bacc, mybir
from concourse.bass_utils import run_bass_kernel_spmd

F16 = mybir.dt.float16
F32 = mybir.dt.float32
AX = mybir.AxisListType
ALU = mybir.AluOpType
ACT = mybir.ActivationFunctionType

M = 128          # rows of x
K = 4096         # contraction
N = 11008        # out features
NCORES = 8
NLOC = N // NCORES           # 1376 columns per core
BLOCK_M, BLOCK_K = 32, 64
NBM, NBK = M // BLOCK_M, K // BLOCK_K   # 4 row blocks, 64 k blocks
KEEP = 32                               # k blocks kept per row block
NKT = K // 128                          # 32 k tiles of 128
N_TILES = [(0, 512), (512, 512), (1024, 352)]   # n-tile offsets/sizes
# W streaming chunks in k-tiles: big in the middle (11KB descriptors),
# small at both ends (quick first matmul, short tail)
W_CHUNKS = [2, 4, 4, 4, 4, 4, 4, 4, 1, 1]
assert sum(W_CHUNKS) == NKT
N_WARM = 16
CPW = 128 + 128 + NLOC   # packed const width: ident | ET | bias


def _program(ctx: ExitStack, tc: tile.TileContext, ins, outs):
    nc = tc.nc
    x_d, w_d, cp_d, e_d = ins
    (o_d,) = outs

    const = ctx.enter_context(tc.tile_pool(name="const", bufs=1))
    mk = ctx.enter_context(tc.tile_pool(name="mk", bufs=1))
    wpool = ctx.enter_context(tc.tile_pool(name="wpool", bufs=1))
    opool = ctx.enter_context(tc.tile_pool(name="opool", bufs=1))
    psum = ctx.enter_context(tc.tile_pool(name="psum", bufs=1, space="PSUM"))

    # ---- first W chunks lead the stream, then x (which gates the top-k
    # mask), then the rest of W
    wpend = []            # (chunk_idx, nkt, kt0, engine)
    kt0 = 0
    for ci, nkt_c in enumerate(W_CHUNKS):
        wpend.append((ci, nkt_c, kt0, nc.sync if ci % 2 == 0 else nc.scalar))
        kt0 += nkt_c

    def issue_w(ci, nkt_c, kt0, eng):
        w_t = wpool.tile([128, nkt_c * NLOC], F16, name=f"wch{ci}")
        eng.dma_start(w_t[:], w_d[:, kt0 * NLOC:(kt0 + nkt_c) * NLOC])
        return [(w_t, i * NLOC) for i in range(nkt_c)]

    w_tiles = []
    for args in wpend[:2]:
        w_tiles += issue_w(*args)

    NCH = 4
    ks = K // NCH                # 1024 cols per chunk, 2KB runs per partition
    xc = ctx.enter_context(tc.tile_pool(name="xc", bufs=NCH))
    x_tiles = []
    for c in range(NCH):
        x_c = xc.tile([128, ks], F16, name=f"xch{c}", tag="xch")
        (nc.sync if c % 2 == 0 else nc.scalar).dma_start(
            x_c[:], x_d[:, c * ks:(c + 1) * ks])
        x_tiles.append(x_c)

    # ---- consts: one packed f16 DMA (ident | ET | bias) + E (f32)
    cpack = const.tile([128, CPW], F16)
    nc.sync.dma_start(cpack[:], cp_d)
    ident = cpack[:, 0:128]
    et_sb = cpack[0:NBM, 128:256]
    bias_sb = cpack[0:1, 256:256 + NLOC]
    e_sb = const.tile([128, NBM], F32)
    nc.scalar.dma_start(e_sb[:], e_d)

    # ---- rest of the W chunks behind x on the same two queues. Packed
    # layout: w_d[p, kt*NLOC + n] = W[kt*128 + p, n] → per-partition
    # contiguous runs of nkt*2752 bytes per chunk.
    for args in wpend[2:]:
        w_tiles += issue_w(*args)

    # ---- HAM warm-up: junk matmuls so the PE clock gate opens before the
    # GEMM starts (otherwise everything runs at 1.2 GHz)
    warm_sb = mk.tile([128, 512], F16)
    nc.vector.memset(warm_sb[:], 0.0)
    pbanks = [psum.tile([128, 512], F32, name=f"pn{i}", tag=f"pn{i}")
              for i in range(3)]
    for i in range(N_WARM):
        nc.tensor.matmul(pbanks[0][:], lhsT=warm_sb[:, 0:128], rhs=warm_sb[:],
                         start=True, stop=True)

    # ---- per-chunk |x| block partial sums (DVE, as chunks land)
    part_n = mk.tile([128, NBK], F32)
    jc = NBK // NCH              # 16 k-blocks per chunk
    for c in range(NCH):
        # part_n[m, j] = sum_k |x[m, 64 j + k]| over this chunk's j's
        nc.vector.tensor_reduce(
            part_n[:, c * jc:(c + 1) * jc],
            x_tiles[c][:].rearrange("p (j k) -> p j k", k=BLOCK_K),
            axis=AX.X, op=ALU.add, apply_absolute_value=True)

    # ba_ps[b, j] = sum_m E[m, b] * part_n[m, j]  (block sums, b on partitions)
    ba_ps = psum.tile([NBM, NBK], F32, tag="mkps", bufs=2)
    nc.tensor.matmul(ba_ps[:], lhsT=e_sb[:], rhs=part_n[:], start=True, stop=True)

    # mean = sum / 2048 (exact power of two), rounded to f16 like jnp.mean
    ba16 = mk.tile([NBM, NBK], F16)
    nc.vector.tensor_scalar_mul(ba16[:], ba_ps[:], 1.0 / 2048.0)

    # arow[i, b*64+j] = a[b, j] on 64 partitions, via block-diag expand + matmul
    # rhs3[c, b*64+j] = a[c, j] * [c == b]
    rhs3 = mk.tile([NBM, NBM * NBK], F16)
    nc.vector.tensor_tensor(
        rhs3[:].rearrange("c (b j) -> c b j", b=NBM),
        ba16[:].unsqueeze(1).broadcast_to((NBM, NBM, NBK)),
        ident[0:NBM, 0:NBM].unsqueeze(-1).broadcast_to((NBM, NBM, NBK)),
        op=ALU.mult)
    ones4c = mk.tile([NBM, 64], F16)
    nc.vector.memset(ones4c[:], 1.0)
    arow_ps = psum.tile([64, NBM * NBK], F32, tag="mkps", bufs=2)
    nc.tensor.matmul(arow_ps[:], lhsT=ones4c[:], rhs=rhs3[:], start=True, stop=True)
    arow = mk.tile([64, NBM * NBK], F16)
    nc.vector.tensor_copy(arow[:], arow_ps[:])

    # acol[i, b] = a[b, i] via PE transpose
    acol_ps = psum.tile([64, NBM], F16, tag="mkps", bufs=2)
    nc.tensor.transpose(acol_ps[:], ba16[:], ident[0:NBM, 0:NBM])
    acol = mk.tile([64, NBM], F16)
    nc.vector.tensor_copy(acol[:], acol_ps[:])

    # cnt[i, b] = #{j : a[b, j] > a[b, i]};  keep iff cnt < KEEP
    cmp = mk.tile([64, NBM * NBK], F16)
    nc.vector.tensor_tensor(
        cmp[:].rearrange("i (b j) -> i b j", b=NBM),
        arow[:].rearrange("i (b j) -> i b j", b=NBM),
        acol[:].unsqueeze(-1).broadcast_to((64, NBM, NBK)),
        op=ALU.is_gt)
    cnt = mk.tile([64, NBM], F32)
    nc.vector.tensor_reduce(cnt[:], cmp[:].rearrange("i (b j) -> i b j", b=NBM),
                            axis=AX.X, op=ALU.add)
    keep16 = mk.tile([64, NBM], F16)
    nc.vector.tensor_scalar(keep16[:], cnt[:], float(KEEP), None, op0=ALU.is_lt)

    # km[b, j] = keep16[j, b] via PE transpose, then expand to rows:
    # keepx[m, j] = keep for (row-block of m, k-block j) = sum_b ET[b, m]*km[b, j]
    km_ps = psum.tile([NBM, NBK], F16, tag="mkps", bufs=2)
    nc.tensor.transpose(km_ps[:], keep16[:], ident[0:64, 0:64])
    km = mk.tile([NBM, NBK], F16)
    nc.vector.tensor_copy(km[:], km_ps[:])
    keepx_ps = psum.tile([128, NBK], F32, tag="mkps", bufs=2)
    nc.tensor.matmul(keepx_ps[:], lhsT=et_sb, rhs=km[:], start=True, stop=True)
    keepx = mk.tile([128, NBK], F16)
    nc.vector.tensor_copy(keepx[:], keepx_ps[:])

    # ---- mask x, then PE-transpose each 128-wide k tile of each chunk
    TPC = NKT // NCH             # 8 k tiles per chunk
    xmc = ctx.enter_context(tc.tile_pool(name="xmc", bufs=NCH))
    xtpool = ctx.enter_context(tc.tile_pool(name="xtpool", bufs=NKT))
    xt_tiles = []
    for c in range(NCH):
        xm_c = xmc.tile([128, ks], F16, name=f"xmch{c}", tag="xmch")
        (nc.vector if c % 2 == 0 else nc.gpsimd).tensor_tensor(
            xm_c[:].rearrange("p (j k) -> p j k", k=BLOCK_K),
            x_tiles[c][:].rearrange("p (j k) -> p j k", k=BLOCK_K),
            keepx[:, c * jc:(c + 1) * jc].unsqueeze(-1)
                .broadcast_to((128, jc, BLOCK_K)),
            op=ALU.mult)
        for t in range(TPC):
            kt = TPC * c + t
            tp = psum.tile([128, 128], F16, name=f"tp{kt}", tag="tp", bufs=2)
            nc.tensor.transpose(tp[:], xm_c[:, t * 128:(t + 1) * 128], ident)
            xt_t = xtpool.tile([128, 128], F16, name=f"xt{kt}", tag="xt")
            nc.vector.tensor_copy(xt_t[:], tp[:])
            xt_tiles.append(xt_t)

    ones = const.tile([1, 128], F16)
    nc.vector.memset(ones[:], 1.0)

    # ---- main GEMM: out[m, n] = sum_kt xt_kt.T @ w_kt + ones.T @ bias ----
    # bias as the FIRST accumulation into each bank (start=True) so the
    # banks are complete right when the last k-tile matmul lands
    for nt, (n0, nsz) in enumerate(N_TILES):
        nc.tensor.matmul(pbanks[nt][:, :nsz], lhsT=ones[:],
                         rhs=bias_sb[:, n0:n0 + nsz], start=True, stop=False)
    for kt in range(NKT):
        w_t, co = w_tiles[kt]
        for nt, (n0, nsz) in enumerate(N_TILES):
            nc.tensor.matmul(pbanks[nt][:, :nsz],
                             lhsT=xt_tiles[kt][:],
                             rhs=w_t[:, co + n0:co + n0 + nsz],
                             start=False, stop=(kt == NKT - 1))
    out_sb = opool.tile([128, NLOC], F16)
    out_dma = [nc.sync, nc.scalar]
    pi = 0
    for nt, (n0, nsz) in enumerate(N_TILES):
        for half in range(2):
            h0 = n0 + half * (nsz // 2)
            hsz = nsz // 2 if half == 0 else nsz - nsz // 2
            src = pbanks[nt][:, h0 - n0:h0 - n0 + hsz]
            dst = out_sb[:, h0:h0 + hsz]
            if pi % 2 == 0:
                nc.scalar.activation(dst, src, ACT.Copy)
            else:
                nc.vector.tensor_copy(dst, src)
            out_dma[pi % 2].dma_start(o_d[:, h0:h0 + hsz], dst)
            pi += 1


_CACHE = {}


def _build():
    if "nc" in _CACHE:
        return _CACHE["nc"]
    nc = bacc.Bacc("TRN2", target_bir_lowering=False, debug=False,
                   num_devices=NCORES)
    x_d = nc.dram_tensor("x", (M, K), F16, kind="ExternalInput").ap()
    w_d = nc.dram_tensor("w", (128, NKT * NLOC), F16, kind="ExternalInput").ap()
    cp_d = nc.dram_tensor("cpack", (128, CPW), F16, kind="ExternalInput").ap()
    e_d = nc.dram_tensor("E", (M, NBM), F32, kind="ExternalInput").ap()
    o_d = nc.dram_tensor("out", (M, NLOC), F16, kind="ExternalOutput").ap()
    with tile.TileContext(nc) as tc:
        with ExitStack() as ctx:
            _program(ctx, tc, [x_d, w_d, cp_d, e_d], [o_d])
    nc.compile()
    _CACHE["nc"] = nc
    return nc


def _make_in_maps(x2, weight, bias):
    e_np = np.zeros((M, NBM), np.float32)
    for b in range(NBM):
        e_np[b * BLOCK_M:(b + 1) * BLOCK_M, b] = 1.0
    cp_np = np.zeros((M, CPW), np.float16)
    cp_np[:, 0:128] = np.eye(128, dtype=np.float16)
    cp_np[0:NBM, 128:256] = e_np.T.astype(np.float16)

    weight = np.asarray(weight).astype(np.float16, copy=False)
    bias = np.asarray(bias).astype(np.float16, copy=False)
    in_maps = []
    for c in range(NCORES):
        sl = slice(c * NLOC, (c + 1) * NLOC)
        cp_c = cp_np.copy()
        cp_c[0, 256:256 + NLOC] = bias[sl]
        # pack so partition p holds row kt*128+p of every k-tile contiguously
        w_c = weight[:, sl].reshape(NKT, 128, NLOC).transpose(1, 0, 2)
        in_maps.append({
            "x": x2,
            "w": np.ascontiguousarray(w_c).reshape(128, NKT * NLOC),
            "cpack": cp_c,
            "E": e_np,
        })
    return in_maps


def kernel(x: np.ndarray, weight: np.ndarray, bias: np.ndarray) -> np.ndarray:
    x = np.asarray(x)
    weight = np.asarray(weight)
    bias = np.asarray(bias)
    bsz, seq, hidden = x.shape
    assert (bsz, seq, hidden) == (M, 1, K) and weight.shape == (K, N)

    x2 = np.ascontiguousarray(x.reshape(M, K).astype(np.float16, copy=False))
    in_maps = _make_in_maps(x2, weight, bias)
    nc = _build()
    res = run_bass_kernel_spmd(nc, in_maps, core_ids=list(range(NCORES)))
    out = np.concatenate([r["out"] for r in res.results], axis=1)
    return out.reshape(M, 1, N).astype(x.dtype, copy=False)


if __name__ == "__main__":
    rng = np.random.default_rng(0)
    x = rng.standard_normal((M, 1, K)).astype(np.float16)
    w = (rng.standard_normal((K, N)) * 0.01).astype(np.float16)
    b = np.zeros((N,), np.float16)
    out = kernel(x, w, b)
    print(out.shape, out.dtype)


# revision 11
# speedup vs baseline: 1.2422x; 1.2422x over previous
"""Block-sparse top-k masked linear for Trainium2, tensor-parallel over 8 cores.

out = (block_masked x) @ W + bias
  x: (128, 1, 4096) fp16, W: (4096, 11008) fp16, bias: (11008,) fp16
  mask: per (32-row x 64-col) block of x, keep blocks whose mean |x| is
  >= the 32nd-largest of the 64 k-block activations in that row block.

Sharding: column-parallel — each of the 8 cores gets an 11008/8 = 1376
column slice of W and bias; x is replicated; outputs are concatenated.

Schedule: x chunks dispatch first on the two HWDGE queues (they gate the
top-k mask), then one packed f16 const DMA (ident|ET|bias) + E, then the
W stream in big chunks. W is pre-packed on the host to (128, 32*1376):
partition p holds row kt*128+p of every k-tile contiguously, so W chunks
move multi-KB contiguous runs per partition (full HBM rate). The mask is
applied to x rows (per partition-block, per 64-col k-block) before the
PE transposes that feed the GEMM.
"""
from contextlib import ExitStack

import numpy as np

import concourse.bass as bass
import concourse.tile as tile
from concourse import bacc, mybir
from concourse.bass_utils import run_bass_kernel_spmd

F16 = mybir.dt.float16
F32 = mybir.dt.float32
AX = mybir.AxisListType
ALU = mybir.AluOpType
ACT = mybir.ActivationFunctionType

M = 128          # rows of x
K = 4096         # contraction
N = 11008        # out features
NCORES = 8
NLOC = N // NCORES           # 1376 columns per core
BLOCK_M, BLOCK_K = 32, 64
NBM, NBK = M // BLOCK_M, K // BLOCK_K   # 4 row blocks, 64 k blocks
KEEP = 32                               # k blocks kept per row block
NKT = K // 128                          # 32 k tiles of 128
N_TILES = [(0, 512), (512, 512), (1024, 352)]   # n-tile offsets/sizes
# W streaming chunks in k-tiles: big in the middle (11KB descriptors),
# small at both ends (quick first matmul, short tail)
W_CHUNKS = [2, 4, 4, 4, 4, 4, 4, 4, 1, 1]
assert sum(W_CHUNKS) == NKT
N_WARM = 16
CPW = 128 + 128 + NLOC   # packed const width: ident | ET | bias


def _program(ctx: ExitStack, tc: tile.TileContext, ins, outs):
    nc = tc.nc
    x_d, w_d, cp_d, e_d = ins
    (o_d,) = outs

    const = ctx.enter_context(tc.tile_pool(name="const", bufs=1))
    mk = ctx.enter_context(tc.tile_pool(name="mk", bufs=1))
    wpool = ctx.enter_context(tc.tile_pool(name="wpool", bufs=1))
    opool = ctx.enter_context(tc.tile_pool(name="opool", bufs=1))
    psum = ctx.enter_context(tc.tile_pool(name="psum", bufs=1, space="PSUM"))

    # ---- x first on both HWDGE queues: it gates the top-k mask, and the
    # first 8 dispatches (x0-3, cpack, E, w0, w1) exactly fill the 8 DMA
    # semaphore lanes, so none of them is guarded behind a W transfer
    NCH = 4
    ks = K // NCH                # 1024 cols per chunk, 2KB runs per partition
    xc = ctx.enter_context(tc.tile_pool(name="xc", bufs=NCH))
    x_tiles = []
    for c in range(NCH):
        x_c = xc.tile([128, ks], F16, name=f"xch{c}", tag="xch")
        (nc.sync if c % 2 == 0 else nc.scalar).dma_start(
            x_c[:], x_d[:, c * ks:(c + 1) * ks])
        x_tiles.append(x_c)

    # ---- consts: one packed f16 DMA (ident | ET | bias) + E (f32)
    cpack = const.tile([128, CPW], F16)
    nc.sync.dma_start(cpack[:], cp_d)
    ident = cpack[:, 0:128]
    et_sb = cpack[0:NBM, 128:256]
    bias_sb = cpack[0:1, 256:256 + NLOC]
    e_sb = const.tile([128, NBM], F32)
    nc.scalar.dma_start(e_sb[:], e_d)

    # ---- W chunks behind x on the same two queues. Packed layout:
    # w_d[p, kt*NLOC + n] = W[kt*128 + p, n] → per-partition contiguous
    # runs of nkt*2752 bytes per chunk.
    w_tiles = []          # per k-tile: (chunk_tile, col offset)
    kt0 = 0
    for ci, nkt_c in enumerate(W_CHUNKS):
        w_t = wpool.tile([128, nkt_c * NLOC], F16, name=f"wch{ci}")
        (nc.sync if ci % 2 == 0 else nc.scalar).dma_start(
            w_t[:], w_d[:, kt0 * NLOC:(kt0 + nkt_c) * NLOC])
        for i in range(nkt_c):
            w_tiles.append((w_t, i * NLOC))
        kt0 += nkt_c

    # ---- HAM warm-up: junk matmuls so the PE clock gate opens before the
    # GEMM starts (otherwise everything runs at 1.2 GHz)
    warm_sb = mk.tile([128, 512], F16)
    nc.vector.memset(warm_sb[:], 0.0)
    pbanks = [psum.tile([128, 512], F32, name=f"pn{i}", tag=f"pn{i}")
              for i in range(3)]
    for i in range(N_WARM):
        nc.tensor.matmul(pbanks[0][:], lhsT=warm_sb[:, 0:128], rhs=warm_sb[:],
                         start=True, stop=True)

    # ---- per-chunk |x| block partial sums (DVE, as chunks land)
    part_n = mk.tile([128, NBK], F32)
    jc = NBK // NCH              # 16 k-blocks per chunk
    for c in range(NCH):
        # part_n[m, j] = sum_k |x[m, 64 j + k]| over this chunk's j's
        nc.vector.tensor_reduce(
            part_n[:, c * jc:(c + 1) * jc],
            x_tiles[c][:].rearrange("p (j k) -> p j k", k=BLOCK_K),
            axis=AX.X, op=ALU.add, apply_absolute_value=True)

    # ba_ps[b, j] = sum_m E[m, b] * part_n[m, j]  (block sums, b on partitions)
    ba_ps = psum.tile([NBM, NBK], F32, tag="mkps", bufs=2)
    nc.tensor.matmul(ba_ps[:], lhsT=e_sb[:], rhs=part_n[:], start=True, stop=True)

    # mean = sum / 2048 (exact power of two), rounded to f16 like jnp.mean
    ba16 = mk.tile([NBM, NBK], F16)
    nc.vector.tensor_scalar_mul(ba16[:], ba_ps[:], 1.0 / 2048.0)

    # arow[i, b*64+j] = a[b, j] on 64 partitions, via block-diag expand + matmul
    # rhs3[c, b*64+j] = a[c, j] * [c == b]
    rhs3 = mk.tile([NBM, NBM * NBK], F16)
    nc.vector.tensor_tensor(
        rhs3[:].rearrange("c (b j) -> c b j", b=NBM),
        ba16[:].unsqueeze(1).broadcast_to((NBM, NBM, NBK)),
        ident[0:NBM, 0:NBM].unsqueeze(-1).broadcast_to((NBM, NBM, NBK)),
        op=ALU.mult)
    ones4c = mk.tile([NBM, 64], F16)
    nc.vector.memset(ones4c[:], 1.0)
    arow_ps = psum.tile([64, NBM * NBK], F32, tag="mkps", bufs=2)
    nc.tensor.matmul(arow_ps[:], lhsT=ones4c[:], rhs=rhs3[:], start=True, stop=True)
    arow = mk.tile([64, NBM * NBK], F16)
    nc.vector.tensor_copy(arow[:], arow_ps[:])

    # acol[i, b] = a[b, i] via PE transpose
    acol_ps = psum.tile([64, NBM], F16, tag="mkps", bufs=2)
    nc.tensor.transpose(acol_ps[:], ba16[:], ident[0:NBM, 0:NBM])
    acol = mk.tile([64, NBM], F16)
    nc.vector.tensor_copy(acol[:], acol_ps[:])

    # cnt[i, b] = #{j : a[b, j] > a[b, i]};  keep iff cnt < KEEP
    cmp = mk.tile([64, NBM * NBK], F16)
    nc.vector.tensor_tensor(
        cmp[:].rearrange("i (b j) -> i b j", b=NBM),
        arow[:].rearrange("i (b j) -> i b j", b=NBM),
        acol[:].unsqueeze(-1).broadcast_to((64, NBM, NBK)),
        op=ALU.is_gt)
    cnt = mk.tile([64, NBM], F32)
    nc.vector.tensor_reduce(cnt[:], cmp[:].rearrange("i (b j) -> i b j", b=NBM),
                            axis=AX.X, op=ALU.add)
    keep16 = mk.tile([64, NBM], F16)
    nc.vector.tensor_scalar(keep16[:], cnt[:], float(KEEP), None, op0=ALU.is_lt)

    # km[b, j] = keep16[j, b] via PE transpose, then expand to rows:
    # keepx[m, j] = keep for (row-block of m, k-block j) = sum_b ET[b, m]*km[b, j]
    km_ps = psum.tile([NBM, NBK], F16, tag="mkps", bufs=2)
    nc.tensor.transpose(km_ps[:], keep16[:], ident[0:64, 0:64])
    km = mk.tile([NBM, NBK], F16)
    nc.vector.tensor_copy(km[:], km_ps[:])
    keepx_ps = psum.tile([128, NBK], F32, tag="mkps", bufs=2)
    nc.tensor.matmul(keepx_ps[:], lhsT=et_sb, rhs=km[:], start=True, stop=True)
    keepx = mk.tile([128, NBK], F16)
    nc.vector.tensor_copy(keepx[:], keepx_ps[:])

    # ---- mask x, then PE-transpose each 128-wide k tile of each chunk
    TPC = NKT // NCH             # 8 k tiles per chunk
    xmc = ctx.enter_context(tc.tile_pool(name="xmc", bufs=NCH))
    xtpool = ctx.enter_context(tc.tile_pool(name="xtpool", bufs=NKT))
    xt_tiles = []
    for c in range(NCH):
        xm_c = xmc.tile([128, ks], F16, name=f"xmch{c}", tag="xmch")
        (nc.vector if c % 2 == 0 else nc.gpsimd).tensor_tensor(
            xm_c[:].rearrange("p (j k) -> p j k", k=BLOCK_K),
            x_tiles[c][:].rearrange("p (j k) -> p j k", k=BLOCK_K),
            keepx[:, c * jc:(c + 1) * jc].unsqueeze(-1)
                .broadcast_to((128, jc, BLOCK_K)),
            op=ALU.mult)
        for t in range(TPC):
            kt = TPC * c + t
            tp = psum.tile([128, 128], F16, name=f"tp{kt}", tag="tp", bufs=2)
            nc.tensor.transpose(tp[:], xm_c[:, t * 128:(t + 1) * 128], ident)
            xt_t = xtpool.tile([128, 128], F16, name=f"xt{kt}", tag="xt")
            nc.vector.tensor_copy(xt_t[:], tp[:])
            xt_tiles.append(xt_t)

    ones = const.tile([1, 128], F16)
    nc.vector.memset(ones[:], 1.0)

    # ---- main GEMM: out[m, n] = sum_kt xt_kt.T @ w_kt + ones.T @ bias ----
    # bias as the FIRST accumulation into each bank (start=True) so the
    # banks are complete right when the last k-tile matmul lands
    for nt, (n0, nsz) in enumerate(N_TILES):
        nc.tensor.matmul(pbanks[nt][:, :nsz], lhsT=ones[:],
                         rhs=bias_sb[:, n0:n0 + nsz], start=True, stop=False)
    for kt in range(NKT):
        w_t, co = w_tiles[kt]
        for nt, (n0, nsz) in enumerate(N_TILES):
            nc.tensor.matmul(pbanks[nt][:, :nsz],
                             lhsT=xt_tiles[kt][:],
                             rhs=w_t[:, co + n0:co + n0 + nsz],
                             start=False, stop=(kt == NKT - 1))
    out_sb = opool.tile([128, NLOC], F16)
    out_dma = [nc.sync, nc.scalar]
    pi = 0
    for nt, (n0, nsz) in enumerate(N_TILES):
        for half in range(2):
            h0 = n0 + half * (nsz // 2)
            hsz = nsz // 2 if half == 0 else nsz - nsz // 2
            src = pbanks[nt][:, h0 - n0:h0 - n0 + hsz]
            dst = out_sb[:, h0:h0 + hsz]
            if pi % 2 == 0:
                nc.scalar.activation(dst, src, ACT.Copy)
            else:
                nc.vector.tensor_copy(dst, src)
            out_dma[pi % 2].dma_start(o_d[:, h0:h0 + hsz], dst)
            pi += 1


_CACHE = {}


def _build():
    if "nc" in _CACHE:
        return _CACHE["nc"]
    nc = bacc.Bacc("TRN2", target_bir_lowering=False, debug=False,
                   num_devices=NCORES)
    x_d = nc.dram_tensor("x", (M, K), F16, kind="ExternalInput").ap()
    w_d = nc.dram_tensor("w", (128, NKT * NLOC), F16, kind="ExternalInput").ap()
    cp_d = nc.dram_tensor("cpack", (128, CPW), F16, kind="ExternalInput").ap()
    e_d = nc.dram_tensor("E", (M, NBM), F32, kind="ExternalInput").ap()
    o_d = nc.dram_tensor("out", (M, NLOC), F16, kind="ExternalOutput").ap()
    with tile.TileContext(nc) as tc:
        with ExitStack() as ctx:
            _program(ctx, tc, [x_d, w_d, cp_d, e_d], [o_d])
    nc.compile()
    _CACHE["nc"] = nc
    return nc


def _make_in_maps(x2, weight, bias):
    e_np = np.zeros((M, NBM), np.float32)
    for b in range(NBM):
        e_np[b * BLOCK_M:(b + 1) * BLOCK_M, b] = 1.0
    cp_np = np.zeros((M, CPW), np.float16)
    cp_np[:, 0:128] = np.eye(128, dtype=np.float16)
    cp_np[0:NBM, 128:256] = e_np.T.astype(np.float16)

    weight = np.asarray(weight).astype(np.float16, copy=False)
    bias = np.asarray(bias).astype(np.float16, copy=False)
    in_maps = []
    for c in range(NCORES):
        sl = slice(c * NLOC, (c + 1) * NLOC)
        cp_c = cp_np.copy()
        cp_c[0, 256:256 + NLOC] = bias[sl]
        # pack so partition p holds row kt*128+p of every k-tile contiguously
        w_c = weight[:, sl].reshape(NKT, 128, NLOC).transpose(1, 0, 2)
        in_maps.append({
            "x": x2,
            "w": np.ascontiguousarray(w_c).reshape(128, NKT * NLOC),
            "cpack": cp_c,
            "E": e_np,
        })
    return in_maps


def kernel(x: np.ndarray, weight: np.ndarray, bias: np.ndarray) -> np.ndarray:
    x = np.asarray(x)
    weight = np.asarray(weight)
    bias = np.asarray(bias)
    bsz, seq, hidden = x.shape
    assert (bsz, seq, hidden) == (M, 1, K) and weight.shape == (K, N)

    x2 = np.ascontiguousarray(x.reshape(M, K).astype(np.float16, copy=False))
    in_maps = _make_in_maps(x2, weight, bias)
    nc = _build()
    res = run_bass_kernel_spmd(nc, in_maps, core_ids=list(range(NCORES)))
    out = np.concatenate([r["out"] for r in res.results], axis=1)
    return out.reshape(M, 1, N).astype(x.dtype, copy=False)


if __name__ == "__main__":
    rng = np.random.default_rng(0)
    x = rng.standard_normal((M, 1, K)).astype(np.float16)
    w = (rng.standard_normal((K, N)) * 0.01).astype(np.float16)
    b = np.zeros((N,), np.float16)
    out = kernel(x, w, b)
    print(out.shape, out.dtype)
